# revision 7
# baseline (speedup 1.0000x reference)
"""AttnBlock (GroupNorm -> single-head 4096-token attention -> proj -> residual)
for Trainium2, SPMD over 8 NeuronCores.

Sharding: data-parallel over batch N=4 (one sample per core-pair); each pair
splits the 4096 queries in half (2048 queries/core). K/V work (GroupNorm +
v projection over all 4096 tokens) is duplicated within a pair - it is
small next to the O(HW^2) attention.

Per-core design (v2 - fp8 DoubleRow attention):
  - Channel-major everywhere: x^T, qW are [C=128 partitions, tokens].
  - GroupNorm folded into the projections (as v1): score[k,q] =
    h[:,k].(M0TA.T @ xqb + bias0)[:,q], h = x*A+B, M0 = wk.T@wq*C^-0.5.
  - Scores bf16: matmul(lhsT=h 128-tok tile, rhs=qW 512-q tile) - 216ns each,
    at the PE stream roofline.
  - P = exp(scores) stored as fp8e5 [C, 32, 512]. exp is split across two
    engines per 3-k-tile group: ACT (native Exp, fp8e5 out) and DVE
    (Schraudolph bitcast: uint8(s*4/ln2 + 59.83) reinterpreted as e5m2,
    max rel err ~12% = e5m2 storage error; error is suppressed ~1e5x by
    wp~1e-5 in the output projection).
  - PV and softmax denominator both run as fp8 DoubleRow matmuls (2 k-tiles
    = 256-row contraction per 216ns instruction, 2x bf16 throughput):
    pv += v[:,2j:2j+2,:].T @ P-pair, den += ones8.T @ P-pair. No DVE folds.
  - v projected to fp8e4 WITHOUT bias: since softmax rows sum to 1, the v
    bias commutes to the output bias: bp' = wp @ bv + bp (host-precomputed).
  - The divide by the denominator commutes past the output projection and is
    applied at the end: out = (wpt.T @ pv) * (1/den) + bp' + x.
  - Head: x DMA in 512-col chunks with bn_stats pipelined per chunk; consts
    on a second (ACT) DMA queue; dummy matmuls warm the PE p-state during
    the stats window.
"""

from contextlib import ExitStack

import numpy as np
import ml_dtypes

import concourse.bass as bass
import concourse.tile as tile
from concourse import bacc, mybir
from concourse import bass_utils

F32 = mybir.dt.float32
BF16 = mybir.dt.bfloat16
FP8E4 = mybir.dt.float8e4
FP8E5 = mybir.dt.float8e5
U8 = mybir.dt.uint8
OP = mybir.AluOpType
ACTF = mybir.ActivationFunctionType
PM = mybir.MatmulPerfMode

C = 128          # channels (= partition count)
HW = 4096        # tokens per sample
NQ = 2048        # queries per core (half a sample)
QT = 512         # query tile (columns per matmul)
KT = 128         # key tile (rows per score matmul)
NKT = HW // KT   # 32 k-tiles
NQT = NQ // QT   # 4 q-tiles
G = 3            # k-tiles per exp instruction (PSUM banks per score tile)
EPS = 1e-5
N_CORES = 8

LN2 = float(np.log(2.0))
A8 = 4.0 / LN2                  # e5m2 Schraudolph scale
B8 = 4.0 * (15.0 - 0.0434)      # e5m2 Schraudolph bias

# group index -> exp engine ('a' = ACT native exp, 'd' = DVE bitcast trick).
# 11 groups of (3,3,...,3,2) k-tiles. DVE groups sit mid-tile so the DVE is
# free at q-tile boundaries for the epilogue (recip + divide) and at the
# start for the previous tile's residual chain. q-tile 0 gives DVE an extra
# group: the ACT carries the q/v evacuations there.
EXP_ENG0 = ['a', 'a', 'd', 'a', 'd', 'a', 'd', 'a', 'd', 'a', 'a']
EXP_ENG = ['a', 'a', 'a', 'd', 'a', 'd', 'a', 'd', 'a', 'a', 'a']


def _emit(ctx: ExitStack, tc: tile.TileContext, d: dict):
    nc = tc.nc

    consts = ctx.enter_context(tc.tile_pool(name="consts", bufs=1))
    big = ctx.enter_context(tc.tile_pool(name="big", bufs=1))
    small = ctx.enter_context(tc.tile_pool(name="small", bufs=2))
    ppool = ctx.enter_context(tc.tile_pool(name="ppool", bufs=2))
    psA = ctx.enter_context(tc.tile_pool(name="psA", bufs=2, space="PSUM"))
    psB = ctx.enter_context(tc.tile_pool(name="psB", bufs=2, space="PSUM"))

    # ---- loads ----
    # xbf in 512-col chunks on the sync (SP) queue so bn_stats pipelines per
    # chunk; consts + xqb on the ACT queue in parallel; xq (residual, first
    # needed by the first epilogue ~25us in) last on the sync queue.
    xbf = big.tile([C, HW], BF16)
    for j in range(8):
        nc.sync.dma_start(xbf[:, j * 512:(j + 1) * 512],
                          d["xbf"][:, j * 512:(j + 1) * 512])
    M0T = consts.tile([C, C], BF16)
    wvt = consts.tile([C, C], BF16)
    wpt = consts.tile([C, C], BF16)
    ones8 = consts.tile([C, 2, C], FP8E4)
    oh1 = consts.tile([C, 32], F32)
    oh2 = consts.tile([32, C], F32)
    for name, t in (("M0T", M0T), ("wvt", wvt), ("wpt", wpt),
                    ("ones8", ones8), ("oh1", oh1), ("oh2", oh2)):
        nc.scalar.dma_start(t, d[name][:])
    c0 = consts.tile([C, 1], F32)
    bp2 = consts.tile([C, 1], F32)
    gns = consts.tile([C, 1], F32)
    gnb = consts.tile([C, 1], F32)
    for name, t in (("c0", c0), ("bp2", bp2), ("gns", gns), ("gnb", gnb)):
        nc.scalar.dma_start(t, d[name][:])
    xqb = big.tile([C, NQ], BF16)
    nc.scalar.dma_start(xqb, d["xqb"][:])
    xq = big.tile([C, NQ], F32)
    nc.sync.dma_start(xq, d["xq"][:])

    # ---- PE p-state warmup: dummy matmuls on the first-landed xbf chunk ----
    # (junk results into a psB slot that is released before real use). These
    # have no other consumers; they keep the PE clocking up while GN stats run.
    warm = psB.tile([C, QT], F32, tag="mm")
    NWARM = 20
    for i in range(NWARM):
        nc.tensor.matmul(warm, lhsT=xbf[:, 0:128], rhs=xbf[:, 0:512],
                         start=(i == 0), stop=(i == NWARM - 1),
                         skip_group_check=True)

    # ---- GroupNorm stats (32 groups of 4 channels over all HW) ----
    SD = nc.vector.BN_STATS_DIM
    stats = small.tile([C, 8, SD], F32)
    for j in range(8):
        nc.vector.bn_stats(out=stats[:, j, :], in_=xbf[:, j * 512:(j + 1) * 512])
    mv = small.tile([C, nc.vector.BN_AGGR_DIM], F32)  # per-channel [mean, var]
    nc.vector.bn_aggr(out=mv, in_=stats)

    # rowstats = [mean_c, E[x^2]_c]
    rowstats = small.tile([C, 2], F32)
    nc.vector.tensor_copy(rowstats[:, 0:1], mv[:, 0:1])
    nc.vector.scalar_tensor_tensor(rowstats[:, 1:2], mv[:, 0:1], mv[:, 0:1],
                                   mv[:, 1:2], op0=OP.mult, op1=OP.add)

    # group-fold across partitions then broadcast back, via one-hot matmuls:
    # gsum[g,s] = sum_j 0.25*rowstats[4g+j,s]; cstat[4g+j,s] = gsum[g,s]
    gps = psB.tile([C, QT], F32, tag="mm")
    nc.tensor.matmul(gps[0:32, 0:2], lhsT=oh1, rhs=rowstats[:],
                     start=True, stop=True)
    gsb = small.tile([32, 2], F32)
    nc.vector.tensor_copy(gsb, gps[0:32, 0:2])
    cps = psB.tile([C, QT], F32, tag="mm")
    nc.tensor.matmul(cps[0:C, 0:2], lhsT=oh2, rhs=gsb[:], start=True, stop=True)
    cstat = small.tile([C, 2], F32)  # [mean_c, E2_c] (group-folded)
    nc.vector.tensor_copy(cstat, cps[0:C, 0:2])

    # negvar = mean^2 - E2 ; rstd = rsqrt(-negvar + eps) on ACT
    negvar = small.tile([C, 1], F32)
    nc.vector.scalar_tensor_tensor(negvar, cstat[:, 0:1], cstat[:, 0:1],
                                   cstat[:, 1:2], op0=OP.mult, op1=OP.subtract)
    epst = small.tile([C, 1], F32)
    nc.vector.memset(epst, EPS)
    gsq = small.tile([C, 1], F32)
    nc.scalar.activation(gsq, negvar, ACTF.Sqrt, bias=epst[:, 0:1], scale=-1.0)
    rstd = small.tile([C, 1], F32)
    nc.vector.reciprocal(rstd, gsq)
    # dummy exp pulls the exp ACT table load into the pre-stream window
    junk = small.tile([C, 1], F32)
    nc.scalar.activation(junk, epst, ACTF.Exp)

    # affine fold: A = rstd*gn_scale, B = gn_bias - mean*A
    A = small.tile([C, 1], F32)
    B = small.tile([C, 1], F32)
    nc.vector.tensor_mul(A, rstd, gns)
    nc.vector.tensor_mul(B, cstat[:, 0:1], A)
    nc.vector.tensor_sub(B, gnb, B)

    # combined q-projection: M0TA = M0T * A rows; bias0 = M0T.T @ B + c0
    Bb = small.tile([C, 1], BF16)
    nc.vector.tensor_copy(Bb, B)
    M0TA = consts.tile([C, C], BF16)
    nc.vector.tensor_scalar_mul(M0TA, M0T, A[:, 0:1])
    b0p = psB.tile([C, QT], F32, tag="mm")
    nc.tensor.matmul(b0p[0:C, 0:1], lhsT=M0T, rhs=Bb[:, 0:1], start=True, stop=True)
    bias0 = small.tile([C, 1], F32)
    nc.vector.tensor_add(bias0, b0p[0:C, 0:1], c0)

    # h (= x*A + B) for the score lhsT and the v projection, in 512-token
    # chunks so the first score/v matmuls are unblocked quickly
    h = big.tile([C, HW], BF16)
    for j in range(8):
        nc.vector.tensor_scalar(h[:, j * 512:(j + 1) * 512],
                                xbf[:, j * 512:(j + 1) * 512],
                                A[:, 0:1], B[:, 0:1], op0=OP.mult, op1=OP.add)

    # ---- projections ----
    qW = big.tile([C, NQ], BF16)
    v = big.tile([C, NKT, C], FP8E4)  # [token-in-tile, k-tile, channel]

    def q_tile(base, n):
        ps = psA.tile([C, G, QT], F32, tag="s")
        for i in range(n):
            j = base + i
            nc.tensor.matmul(ps[:, i, :], lhsT=M0TA, rhs=xqb[:, j * QT:(j + 1) * QT],
                             start=True, stop=True)
        nc.scalar.activation(qW[:, base * QT:(base + n) * QT],
                             ps[:, 0:n, :].rearrange("c a b -> c (a b)"),
                             ACTF.Identity, bias=bias0[:, 0:1])

    def v_tile(base, eng):
        # 4 token-tiles of 128 columns packed per PSUM bank; evac is a pure
        # fp8e4 downcast (no bias - folded into bp' on host).
        ps = psA.tile([C, G, QT], F32, tag="s")
        for i in range(4):
            nc.tensor.matmul(ps[:, 0, i * C:(i + 1) * C],
                             lhsT=h[:, (base + i) * KT:(base + i + 1) * KT],
                             rhs=wvt, start=(i == 0), stop=(i == 3))
        dst = v[:, base:base + 4, :]
        if eng == 'a':
            nc.scalar.activation(dst, ps[:, 0, :].rearrange("c (f k) -> c f k", k=C),
                                 ACTF.Identity)
        else:
            nc.vector.tensor_copy(dst, ps[:, 0, :].rearrange("c (f k) -> c f k", k=C))

    # q-tile 0's queries evacuated first (a single 512-col evac unblocks the
    # score stream); the remaining three tiles follow.
    q_tile(0, 1)
    q_tile(1, 3)
    v_tile(0, 'a')

    # ---- attention ----
    def epilogue_b(qt, obu):
        # out-projection of the already-divided pv, then bias' + residual.
        # Flushed at the END of the next q-tile so its PSUM slot wait (on
        # that tile's pv) resolves instantly.
        ops_ = psB.tile([C, QT], F32, tag="mm")
        nc.tensor.matmul(ops_, lhsT=wpt, rhs=obu, start=True, stop=True)
        res = small.tile([C, QT], F32, tag="res")
        nc.vector.scalar_tensor_tensor(res, ops_[:], bp2[:, 0:1],
                                       xq[:, qt * QT:(qt + 1) * QT],
                                       op0=OP.add, op1=OP.add)
        for k in range(2):
            sl = slice(qt * QT + k * (QT // 2), qt * QT + (k + 1) * (QT // 2))
            nc.sync.dma_start(d["out"][:, sl], res[:, k * (QT // 2):(k + 1) * (QT // 2)])

    def run_qtile(qt, P, P8u, pv, dps, st):
        qs = qW[:, qt * QT:(qt + 1) * QT]
        pattern = EXP_ENG0 if qt == 0 else EXP_ENG
        pair = 0  # next DoubleRow pair (of k-tiles) to multiply
        g0 = 0
        for gi, eng in enumerate(pattern):
            if qt == 0 and 1 <= gi <= 7:
                # interleave the remaining v projections with the early
                # score groups; chunk c is ready well before pair 2c needs it
                v_tile(gi * 4, 'a' if gi % 2 == 0 else 'd')
            n = min(G, NKT - g0)
            sps = psA.tile([C, G, QT], F32, tag="s")
            for i in range(n):
                kt = g0 + i
                nc.tensor.matmul(sps[:, i, :],
                                 lhsT=h[:, kt * KT:(kt + 1) * KT], rhs=qs,
                                 start=True, stop=True)
            if eng == 'a':
                nc.scalar.activation(P[:, g0:g0 + n, :], sps[:, 0:n, :], ACTF.Exp)
            else:
                nc.vector.tensor_scalar(P8u[:, g0:g0 + n, :], sps[:, 0:n, :],
                                        A8, B8, op0=OP.mult, op1=OP.add)
            g0 += n
            # DoubleRow PV + denominator for every fully-exp'd pair
            while (pair + 1) * 2 <= g0:
                j = pair
                nc.tensor.matmul(pv, lhsT=v[:, 2 * j:2 * j + 2, :],
                                 rhs=P[:, 2 * j:2 * j + 2, :],
                                 start=(j == 0), stop=(j == NKT // 2 - 1),
                                 perf_mode=PM.DoubleRow)
                nc.tensor.matmul(dps, lhsT=ones8, rhs=P[:, 2 * j:2 * j + 2, :],
                                 start=(j == 0), stop=(j == NKT // 2 - 1),
                                 perf_mode=PM.DoubleRow)
                pair += 1

        # epilogue head on DVE: reciprocal of the denominator, then the
        # divide folded into the pv evacuation (column scaling commutes
        # with the output projection)
        rd = small.tile([C, QT], F32, tag="rd")
        nc.vector.reciprocal_approx_fast(rd, dps[:])
        obu = small.tile([C, QT], BF16, tag="obu")
        nc.vector.tensor_mul(obu, pv[:], rd)
        if st["pending"] is not None:
            epilogue_b(*st["pending"])
        st["pending"] = (qt, obu)

    st = {"pending": None}
    for qt in range(NQT):
        P = ppool.tile([C, NKT, QT], FP8E5, tag="P")
        P8u = P.bitcast(U8)
        pv = psB.tile([C, QT], F32, tag="mm")
        dps = psB.tile([C, QT], F32, tag="mm")
        run_qtile(qt, P, P8u, pv, dps, st)
    epilogue_b(*st["pending"])


_CACHE = {}


def _build():
    if "nc" in _CACHE:
        return _CACHE["nc"], _CACHE["d"]
    nc = bacc.Bacc("TRN2", target_bir_lowering=False, debug=False)
    d = {}
    d["xbf"] = nc.dram_tensor("xbf", [C, HW], BF16, kind="ExternalInput").ap()
    d["xqb"] = nc.dram_tensor("xqb", [C, NQ], BF16, kind="ExternalInput").ap()
    d["xq"] = nc.dram_tensor("xq", [C, NQ], F32, kind="ExternalInput").ap()
    for w in ("M0T", "wvt", "wpt"):
        d[w] = nc.dram_tensor(w, [C, C], BF16, kind="ExternalInput").ap()
    d["ones8"] = nc.dram_tensor("ones8", [C, 2, C], FP8E4, kind="ExternalInput").ap()
    d["oh1"] = nc.dram_tensor("oh1", [C, 32], F32, kind="ExternalInput").ap()
    d["oh2"] = nc.dram_tensor("oh2", [32, C], F32, kind="ExternalInput").ap()
    for b in ("c0", "bp2", "gns", "gnb"):
        d[b] = nc.dram_tensor(b, [C, 1], F32, kind="ExternalInput").ap()
    d["out"] = nc.dram_tensor("out", [C, NQ], F32, kind="ExternalOutput").ap()

    with ExitStack() as ctx:
        tc = ctx.enter_context(tile.TileContext(nc))
        _emit(ctx, tc, d)
    nc.compile()
    _CACHE["nc"] = nc
    _CACHE["d"] = d
    return nc, d


def make_in_maps(x, gn_scale, gn_bias, wq, bq, wk, bk, wv, bv, wp, bp):
    """Build the 8 per-core input dicts from the full problem inputs."""
    f32 = np.float32
    bf16 = ml_dtypes.bfloat16
    e4 = ml_dtypes.float8_e4m3fn
    s = f32(C) ** f32(-0.5)
    wq = np.asarray(wq, dtype=f32); wk = np.asarray(wk, dtype=f32)
    wp_ = np.asarray(wp, dtype=f32); bv_ = np.asarray(bv, dtype=f32)
    base = {
        "M0T": np.ascontiguousarray((wq.T @ wk * s).astype(bf16)),
        "wvt": np.ascontiguousarray(np.asarray(wv).T.astype(bf16)),
        "wpt": np.ascontiguousarray(wp_.T.astype(bf16)),
        "ones8": np.ones((C, 2, C), e4),
        "oh1": (np.equal.outer(np.arange(C) // 4, np.arange(32)) * 0.25).astype(f32),
        "oh2": np.equal.outer(np.arange(32), np.arange(C) // 4).astype(f32),
        "c0": (wk.T @ (np.asarray(bq) * s)).astype(f32).reshape(C, 1),
        "bp2": (wp_ @ bv_ + np.asarray(bp, dtype=f32)).astype(f32).reshape(C, 1),
        "gns": np.asarray(gn_scale).astype(f32).reshape(C, 1),
        "gnb": np.asarray(gn_bias).astype(f32).reshape(C, 1),
    }
    in_maps = []
    x = np.asarray(x)
    for core in range(N_CORES):
        n, half = core // 2, core % 2
        xt = np.ascontiguousarray(x[n].reshape(C, HW).astype(f32))
        xbf = xt.astype(bf16)
        in_maps.append({
            **base,
            "xbf": xbf,
            "xqb": np.ascontiguousarray(xbf[:, half * NQ:(half + 1) * NQ]),
            "xq": np.ascontiguousarray(xt[:, half * NQ:(half + 1) * NQ]),
        })
    return in_maps


def assemble(results, x):
    out = np.empty(x.shape, dtype=np.float32)
    for core in range(N_CORES):
        n, half = core // 2, core % 2
        out[n].reshape(C, HW)[:, half * NQ:(half + 1) * NQ] = results[core]["out"]
    return out


def kernel(x, gn_scale, gn_bias, wq, bq, wk, bk, wv, bv, wp, bp, **run_kwargs):
    nc, _ = _build()
    in_maps = make_in_maps(x, gn_scale, gn_bias, wq, bq, wk, bk, wv, bv, wp, bp)
    r = bass_utils.run_bass_kernel_spmd(nc, in_maps, core_ids=list(range(N_CORES)),
                                        **run_kwargs)
    kernel.last_results = r
    return assemble(r.results, np.asarray(x))
